# revision 61
# baseline (speedup 1.0000x reference)
"""Trainium2 Bass kernel for the BDH dense-transformer problem.

Sharding: 8 cores = 4 heads x 2 sequence-halves. Each core owns one head and
512 of the 1024 sequence rows ("own" rows live at permuted positions [0,512)).
Core c = (head h=c//2, group g=c%2); group 1 sees all T-indexed data with the
two 512-halves swapped so the program is identical on every core (pure SPMD,
per-core differences are input data only).

Schedule (tuned against perfetto traces): one AllReduce per layer (chunked
ARs serialize on the CC stream and lose); rope runs on pc-pair (FD=1024)
DVE ops in 2x fp16 mode; the D=128:192 matmul tails for x_sparse/y_sparse
run pairwise on PE row groups 0 and 64 concurrently (weights + xT1/ykvT1
duplicated into partitions 64:128); the post-AR LN tail is batched per
4-slot chunk (one stats pass, gsel-routed scalar_tensor_tensor combines);
dummy PE matmuls bridge the AR window and the scT->ykv stall to keep the
HAM clock gate warm; constant DMAs are ordered so wave 0 starts within a
few us of kernel entry.

All matmul/elementwise storage is fp16 (PE accumulates fp32 in PSUM);
LayerNorm stats run fp32. Scores matmuls are fp8 DoubleRow.
"""

import math

import numpy as np

P = 128
T = 1024
D = 192
NH = 4
N = 3072
NPAIR = 1536
NPC = 12          # 128-row chunks of the 1536 rope pairs
VOCAB = 256
EPS = 1e-5
N_LAYER = 4
NCORES = 8
HALF = 512
TBLK = T // P     # 8 canonical 128-row blocks
VSH = VOCAB // NCORES  # vocab shard per core
DUMMY_AR = 240    # PE warm-keeper matmuls across the AR+LN window
DUMMY_KV = 70     # PE warm-keepers across the scT->ykv dependency stall
DUMMY_TRICKLE = 0  # PE warm-keepers inside wave/ysp phases
POOL_CAST = False  # fp16->fp8 qr casts on GpSimd (False: DVE writes fp8)

_CACHE = {}


def _get_freqs(n, theta=2.0 ** 16):
    t = np.arange(n, dtype=np.float32)
    q = np.floor(t / 2.0) * 2.0
    return (1.0 / theta ** (q / n) / (2.0 * math.pi)).astype(np.float32)


def _ln_np(x):
    m = x.mean(-1, keepdims=True)
    v = x.var(-1, keepdims=True)
    return (x - m) / np.sqrt(v + EPS)


def build_program(repeat=1):
    key = ("nc", repeat)
    if key in _CACHE:
        return _CACHE[key]

    import concourse.mybir as mybir
    import concourse.tile as tile
    from concourse import bacc

    f16 = mybir.dt.float16
    f32 = mybir.dt.float32
    f8 = mybir.dt.float8e4
    AF = mybir.ActivationFunctionType
    OP = mybir.AluOpType
    AX = mybir.AxisListType

    nc = bacc.Bacc("TRN2", target_bir_lowering=False, debug=False,
                   num_devices=NCORES)

    # ---- I/O ----
    x0_d = nc.dram_tensor("x0", [T, D], f16, kind="ExternalInput")
    wxe_d = nc.dram_tensor("wxe", [D, NPAIR], f16, kind="ExternalInput")
    wxo_d = nc.dram_tensor("wxo", [D, NPAIR], f16, kind="ExternalInput")
    wye_d = nc.dram_tensor("wye", [D, NPAIR], f16, kind="ExternalInput")
    wyo_d = nc.dram_tensor("wyo", [D, NPAIR], f16, kind="ExternalInput")
    ence_d = nc.dram_tensor("ence", [NPAIR, D], f16, kind="ExternalInput")
    enco_d = nc.dram_tensor("enco", [NPAIR, D], f16, kind="ExternalInput")
    cos_d = nc.dram_tensor("cosT", [NPAIR, T], f16, kind="ExternalInput")
    sin_d = nc.dram_tensor("sinT", [NPAIR, T], f16, kind="ExternalInput")
    m0_d = nc.dram_tensor("m0", [P, P], f16, kind="ExternalInput")
    gsel_d = nc.dram_tensor("gsel", [P, 2], f32, kind="ExternalInput")
    lmh_d = nc.dram_tensor("lmh", [D, VSH], f16, kind="ExternalInput")
    ident_d = nc.dram_tensor("ident", [P, P], f16, kind="ExternalInput")
    logits_d = nc.dram_tensor("logits", [T, VSH], f32, kind="ExternalOutput")

    with tile.TileContext(nc) as tc:
        with (
            tc.tile_pool(name="const", bufs=1) as cpool,
            tc.tile_pool(name="state", bufs=1) as spool,
            tc.tile_pool(name="work", bufs=2) as work,
            tc.tile_pool(name="stats", bufs=2) as stp,
            tc.tile_pool(name="psum", bufs=1, space="PSUM") as psp,
            tc.tile_pool(name="dram", bufs=1, space="DRAM") as dpool,
        ):
            # ---- persistent SBUF residents ----
            wxe_a = cpool.tile([P, NPAIR], f16, tag="wxe_a")
            wxe_b = cpool.tile([P, NPAIR], f16, tag="wxe_b")
            wxo_a = cpool.tile([P, NPAIR], f16, tag="wxo_a")
            wxo_b = cpool.tile([P, NPAIR], f16, tag="wxo_b")
            wye_a = cpool.tile([P, NPAIR], f16, tag="wye_a")
            wye_b = cpool.tile([P, NPAIR], f16, tag="wye_b")
            wyo_a = cpool.tile([P, NPAIR], f16, tag="wyo_a")
            wyo_b = cpool.tile([P, NPAIR], f16, tag="wyo_b")
            ence_t = cpool.tile([P, NPC, D], f16, tag="ence")
            enco_t = cpool.tile([P, NPC, D], f16, tag="enco")
            cos_t = cpool.tile([P, NPC, T], f16, tag="cos")
            sin_t = cpool.tile([P, NPC, T], f16, tag="sin")
            m0_t = cpool.tile([P, P], f16, tag="m0")
            gsel_t = cpool.tile([P, 2], f32, tag="gsel")
            eps_t = cpool.tile([P, 1], f32, tag="eps")
            ident_t = cpool.tile([P, P], f16, tag="ident")
            lmh_a = cpool.tile([P, VSH], f16, tag="lmh_a")
            lmh_b = cpool.tile([P, VSH], f16, tag="lmh_b")

            qrE = spool.tile([P, NPC, T], f8, tag="qrE")
            qrO = spool.tile([P, NPC, T], f8, tag="qrO")
            Eown = spool.tile([P, NPC, HALF], f16, tag="Eown")
            Oown = spool.tile([P, NPC, HALF], f16, tag="Oown")
            x16 = spool.tile([P, TBLK, D], f16, tag="x16")
            xT0 = spool.tile([P, T], f16, tag="xT0")
            xT1 = spool.tile([P, T], f16, tag="xT1")
            scT = spool.tile([P, TBLK, HALF], f16, tag="scT")
            ykvT0 = spool.tile([P, HALF], f16, tag="ykvT0")
            ykvT1 = spool.tile([P, HALF], f16, tag="ykvT1")
            XM8 = spool.tile([P, TBLK, D], f16, tag="XM8")

            bounce_in = dpool.tile([T, D], f16)
            bounce_out = dpool.tile([T, D], f16)

            # ---- load constants: first-use order so wave 0 starts early.
            # Scalar queue stays clear after wx so the startup transpose
            # copies (Scalar) are not stuck behind table loads.
            nc.sync.dma_start(ident_t[:, :], ident_d[:, :])
            for cb in range(TBLK):
                (nc.sync if cb % 2 == 0 else nc.scalar).dma_start(
                    x16[:, cb, :], x0_d[cb * P:(cb + 1) * P, :])
            nc.scalar.dma_start(wxe_a[:, :], wxe_d[0:P, :])
            nc.scalar.dma_start(wxe_b[0:64, :], wxe_d[P:D, :])
            nc.scalar.dma_start(wxe_b[64:128, :], wxe_d[P:D, :])
            nc.scalar.dma_start(wxo_a[:, :], wxo_d[0:P, :])
            nc.scalar.dma_start(wxo_b[0:64, :], wxo_d[P:D, :])
            nc.scalar.dma_start(wxo_b[64:128, :], wxo_d[P:D, :])
            # cos/sin in pc order on sync+gpsimd only
            for pc in range(NPC):
                nc.sync.dma_start(cos_t[:, pc, :], cos_d[pc * P:(pc + 1) * P, :])
                nc.gpsimd.dma_start(sin_t[:, pc, :], sin_d[pc * P:(pc + 1) * P, :])
            nc.sync.dma_start(m0_t[:, :], m0_d[:, :])
            nc.sync.dma_start(gsel_t[:, :], gsel_d[:, :])
            nc.gpsimd.dma_start(wye_a[:, :], wye_d[0:P, :])
            nc.gpsimd.dma_start(wye_b[0:64, :], wye_d[P:D, :])
            nc.gpsimd.dma_start(wye_b[64:128, :], wye_d[P:D, :])
            nc.gpsimd.dma_start(wyo_a[:, :], wyo_d[0:P, :])
            nc.gpsimd.dma_start(wyo_b[0:64, :], wyo_d[P:D, :])
            nc.gpsimd.dma_start(wyo_b[64:128, :], wyo_d[P:D, :])
            for pc in range(NPC):
                (nc.sync if pc % 2 else nc.gpsimd).dma_start(
                    ence_t[:, pc, :], ence_d[pc * P:(pc + 1) * P, :])
                (nc.gpsimd if pc % 2 else nc.sync).dma_start(
                    enco_t[:, pc, :], enco_d[pc * P:(pc + 1) * P, :])
            nc.gpsimd.dma_start(lmh_a[:, :], lmh_d[0:P, :])
            nc.gpsimd.dma_start(lmh_b[0:64, :], lmh_d[P:D, :])
            nc.gpsimd.dma_start(lmh_b[64:128, :], lmh_d[P:D, :])

            nc.vector.memset(eps_t[:, :], EPS)

            # zero regions of masked score strips (stay zero forever)
            for s in range(1, 4):
                nc.vector.memset(scT[:, s, 0:s * P], 0)

            def ln_vecs(src_ap):
                """LayerNorm stats of src [P,F] -> (r, negmr) [P,1] f32."""
                st = stp.tile([P, 6], f32, tag="bnst")
                nc.vector.bn_stats(st[:, :], src_ap)
                mv = stp.tile([P, 2], f32, tag="bnmv")
                nc.vector.bn_aggr(mv[:, :], st[:, :])
                sd = stp.tile([P, 1], f32, tag="sd")
                nc.scalar.activation(sd[:, :], mv[:, 1:2], AF.Sqrt, bias=eps_t[:, :])
                r = stp.tile([P, 1], f32, tag="r")
                nc.vector.reciprocal(r[:, :], sd[:, :])
                nmr = stp.tile([P, 1], f32, tag="nmr")
                nc.vector.tensor_mul(nmr[:, :], mv[:, 0:1], r[:, :])
                nc.vector.tensor_scalar_mul(nmr[:, :], nmr[:, :], -1.0)
                return r, nmr

            def pe_transpose(src_ap_full, dst0, dst1, col, tagp, evac=None):
                """[128, 192] tile -> dst0[:, col:col+128] (d 0:128) and
                dst1[0:64, col:col+128] (d 128:192) via TensorE transposes.
                evac: engine for the PSUM->SBUF copies (default Vector)."""
                tp0 = psp.tile([P, P], f16, tag="xspO", bufs=2, name=f"tp0_{tagp}")
                nc.tensor.transpose(tp0[:, :], src_ap_full[:, 0:P], ident_t[:, :])
                nc.scalar.copy(dst0[:, col:col + P], tp0[:, :])
                tp1 = psp.tile([P, P], f16, tag="xspO", bufs=2, name=f"tp1_{tagp}")
                nc.tensor.transpose(tp1[0:64, :], src_ap_full[:, P:D], ident_t[:, :])
                nc.scalar.copy(dst1[0:64, col:col + P], tp1[0:64, :])
                nc.scalar.copy(dst1[64:128, col:col + P], tp1[0:64, :])

            def emit_dummies(n, tagn, tag="sc3"):
                """PE warm-keeper matmuls into a throwaway PSUM slot."""
                if n == 0:
                    return
                psD = psp.tile([P, 64], f32, tag=tag, name=f"dum{tagn}",
                               bufs=2 if tag != "sc3" else None)
                for _ in range(n):
                    nc.tensor.matmul(psD[:, :], ident_t[:, :], ident_t[:, 0:64],
                                     start=True, stop=True)

            def chunk_stats4(ap_lo, ap_hi, tagn):
                """Per-slot LN stats over last dim for two [P,2,D] slices.
                Returns rv, nmr [P,4] f32 (slots: lo0, lo1, hi0, hi1)."""
                sm = stp.tile([P, 4], f32, tag="sm4", name=f"sm_{tagn}")
                s2 = stp.tile([P, 4], f32, tag="s24", name=f"s2_{tagn}")
                sq = work.tile([P, 2, D], f16, tag="sq4", bufs=2, name=f"sq_{tagn}")
                nc.vector.tensor_reduce(sm[:, 0:2], ap_lo, AX.X, OP.add)
                nc.scalar.activation(sq[:, :, :], ap_lo, AF.Square)
                nc.vector.tensor_reduce(s2[:, 0:2], sq[:, :, :], AX.X, OP.add)
                sq2 = work.tile([P, 2, D], f16, tag="sq4", bufs=2, name=f"sqb_{tagn}")
                nc.vector.tensor_reduce(sm[:, 2:4], ap_hi, AX.X, OP.add)
                nc.scalar.activation(sq2[:, :, :], ap_hi, AF.Square)
                nc.vector.tensor_reduce(s2[:, 2:4], sq2[:, :, :], AX.X, OP.add)
                mean = stp.tile([P, 4], f32, tag="mean4", name=f"mean_{tagn}")
                nc.vector.tensor_scalar_mul(mean[:, :], sm[:, :], 1.0 / D)
                var = stp.tile([P, 4], f32, tag="var4", name=f"var_{tagn}")
                msq = stp.tile([P, 4], f32, tag="msq4", name=f"msq_{tagn}")
                nc.vector.tensor_mul(msq[:, :], mean[:, :], mean[:, :])
                nc.vector.tensor_scalar_mul(var[:, :], s2[:, :], 1.0 / D)
                nc.vector.tensor_sub(var[:, :], var[:, :], msq[:, :])
                sd = stp.tile([P, 4], f32, tag="sd4", name=f"sd_{tagn}")
                nc.scalar.activation(sd[:, :], var[:, :], AF.Sqrt, bias=eps_t[:, :])
                rv = stp.tile([P, 4], f32, tag="rv4", name=f"rv_{tagn}")
                nc.vector.reciprocal(rv[:, :], sd[:, :])
                nmr = stp.tile([P, 4], f32, tag="nmr4", name=f"nmr_{tagn}")
                nc.vector.tensor_mul(nmr[:, :], mean[:, :], rv[:, :])
                nc.vector.tensor_scalar_mul(nmr[:, :], nmr[:, :], -1.0)
                return rv, nmr

            def chunk_stats8(src3d, tagn):
                """Per-slot LN stats over last dim for a [P,8,D] view."""
                sm = stp.tile([P, TBLK], f32, tag="sm8", name=f"sm_{tagn}")
                s2 = stp.tile([P, TBLK], f32, tag="s28", name=f"s2_{tagn}")
                sq = work.tile([P, TBLK, D], f16, tag="sq8", bufs=1,
                               name=f"sq_{tagn}")
                nc.vector.tensor_reduce(sm[:, :], src3d, AX.X, OP.add)
                nc.scalar.activation(sq[:, :, :], src3d, AF.Square)
                nc.vector.tensor_reduce(s2[:, :], sq[:, :, :], AX.X, OP.add)
                mean = stp.tile([P, TBLK], f32, tag="mean8", name=f"mean_{tagn}")
                nc.vector.tensor_scalar_mul(mean[:, :], sm[:, :], 1.0 / D)
                var = stp.tile([P, TBLK], f32, tag="var8", name=f"var_{tagn}")
                msq = stp.tile([P, TBLK], f32, tag="msq8", name=f"msq_{tagn}")
                nc.vector.tensor_mul(msq[:, :], mean[:, :], mean[:, :])
                nc.vector.tensor_scalar_mul(var[:, :], s2[:, :], 1.0 / D)
                nc.vector.tensor_sub(var[:, :], var[:, :], msq[:, :])
                sd = stp.tile([P, TBLK], f32, tag="sd8", name=f"sd_{tagn}")
                nc.scalar.activation(sd[:, :], var[:, :], AF.Sqrt, bias=eps_t[:, :])
                rv = stp.tile([P, TBLK], f32, tag="rv8", name=f"rv_{tagn}")
                nc.vector.reciprocal(rv[:, :], sd[:, :])
                nmr = stp.tile([P, TBLK], f32, tag="nmr8", name=f"nmr_{tagn}")
                nc.vector.tensor_mul(nmr[:, :], mean[:, :], rv[:, :])
                nc.vector.tensor_scalar_mul(nmr[:, :], nmr[:, :], -1.0)
                return rv, nmr

            def tail_all(li):
                """Readback the AR output, batched LN+residual+LN, new x16
                and xT. Hall slot cb = canonical block cb; XM slots routed
                by gsel (identity for g0 cores, half-swap for g1)."""
                tg = f"l{li}"
                Hall = work.tile([P, TBLK, D], f16, tag="Hall", bufs=1,
                                 name=f"Hall_{tg}")
                for cb in range(TBLK):
                    (nc.sync if cb % 2 == 0 else nc.gpsimd).dma_start(
                        Hall[:, cb, :], bounce_out[cb * P:(cb + 1) * P, :])
                rv, nmr = chunk_stats8(Hall[:, :, :], tg)
                lns = []
                for i in range(TBLK):
                    ln_i = work.tile([P, D], f16, tag=f"lnh{i % 4}", bufs=2,
                                     name=f"ln_{tg}_{i}")
                    nc.scalar.activation(ln_i[:, :], Hall[:, i, :], AF.Identity,
                                         bias=nmr[:, i:i + 1], scale=rv[:, i:i + 1])
                    lns.append(ln_i)
                # XM[j]   = g0*ln[j] + g1*ln[j+4] + x16[j]
                # XM[j+4] = g1*ln[j] + g0*ln[j+4] + x16[j+4]
                for j in range(4):
                    for slot, sa, sb in ((j, 0, 1), (j + 4, 1, 0)):
                        v = work.tile([P, D], f16, tag="vcmb", bufs=2,
                                      name=f"v_{tg}_{slot}")
                        nc.vector.scalar_tensor_tensor(
                            v[:, :], lns[j + 4][:, :], gsel_t[:, sb:sb + 1],
                            x16[:, slot, :], OP.mult, OP.add)
                        nc.vector.scalar_tensor_tensor(
                            XM8[:, slot, :], lns[j][:, :], gsel_t[:, sa:sa + 1],
                            v[:, :], OP.mult, OP.add)
                rv2, nm2 = chunk_stats8(XM8[:, :, :], tg + "f")
                for slot in range(TBLK):
                    nc.scalar.activation(x16[:, slot, :], XM8[:, slot, :],
                                         AF.Identity, bias=nm2[:, slot:slot + 1],
                                         scale=rv2[:, slot:slot + 1])
                    pe_transpose(x16[:, slot, :], xT0, xT1, slot * P,
                                 f"t{tg}_{slot}")

            def load_x_and_transpose():
                for cb in range(TBLK):
                    pe_transpose(x16[:, cb, :], xT0, xT1, cb * P, f"x{cb}")

            def layer(li):
                # ---- waves: x_sparse + rope (pc pairs) + lagged score MMs ----
                LAGP = 2  # pairs

                def sc_mms(s_lo, ps_list, pcp):
                    psl = slice(2 * pcp, 2 * pcp + 2)
                    for si, psS in enumerate(ps_list):
                        s = s_lo + si
                        ssl = slice(s * P, (s + 1) * P)
                        nc.tensor.matmul(psS[:, :], qrE[:, psl, ssl], qrE[:, psl, 0:HALF],
                                         start=(pcp == 0), stop=False,
                                         perf_mode=mybir.MatmulPerfMode.DoubleRow)
                        nc.tensor.matmul(psS[:, :], qrO[:, psl, ssl], qrO[:, psl, 0:HALF],
                                         start=False, stop=(pcp == NPC // 2 - 1),
                                         perf_mode=mybir.MatmulPerfMode.DoubleRow)

                for wave in range(2):
                    s_lo = wave * 4
                    tsl = slice(wave * HALF, (wave + 1) * HALF)
                    ps_list = []
                    for si in range(4):
                        psS = psp.tile([P, HALF], f32, tag=f"sc{si}",
                                       name=f"psS{li}_{s_lo + si}")
                        ps_list.append(psS)
                    for pcp in range(NPC // 2):
                        p2 = slice(2 * pcp, 2 * pcp + 2)
                        if wave == 0:
                            Ew, Ow = None, None
                            Epair = Eown[:, p2, :]
                            Opair = Oown[:, p2, :]
                        else:
                            Ew = work.tile([P, 2, HALF], f16, tag="E1",
                                           name=f"Ew{li}_{pcp}")
                            Ow = work.tile([P, 2, HALF], f16, tag="O1",
                                           name=f"Ow{li}_{pcp}")
                            Epair = Ew[:, :, :]
                            Opair = Ow[:, :, :]
                        pcs0 = slice((2 * pcp) * P, (2 * pcp + 1) * P)
                        pcs1 = slice((2 * pcp + 1) * P, (2 * pcp + 2) * P)
                        psE0 = psp.tile([P, HALF], f32, tag="xspE", bufs=2, name="psE0")
                        psE1 = psp.tile([P, HALF], f32, tag="xspE", bufs=2, name="psE1")
                        psO0 = psp.tile([P, HALF], f32, tag="xspO", bufs=2, name="psO0")
                        psO1 = psp.tile([P, HALF], f32, tag="xspO", bufs=2, name="psO1")
                        nc.tensor.matmul(psE0[:, :], wxe_a[:, pcs0], xT0[:, tsl], start=True, stop=False)
                        nc.tensor.matmul(psO0[:, :], wxo_a[:, pcs0], xT0[:, tsl], start=True, stop=False)
                        nc.tensor.matmul(psE1[:, :], wxe_a[:, pcs1], xT0[:, tsl], start=True, stop=False)
                        nc.tensor.matmul(psO1[:, :], wxo_a[:, pcs1], xT0[:, tsl], start=True, stop=False)
                        # the two K=64 tails run on row groups 0 and 64 concurrently
                        nc.tensor.matmul(psE0[:, :], wxe_b[0:64, pcs0], xT1[0:64, tsl], start=False, stop=True)
                        nc.tensor.matmul(psE1[:, :], wxe_b[64:128, pcs1], xT1[64:128, tsl], start=False, stop=True)
                        nc.tensor.matmul(psO0[:, :], wxo_b[0:64, pcs0], xT1[0:64, tsl], start=False, stop=True)
                        nc.tensor.matmul(psO1[:, :], wxo_b[64:128, pcs1], xT1[64:128, tsl], start=False, stop=True)
                        for k, psEk, psOk in ((0, psE0, psO0), (1, psE1, psO1)):
                            pc = 2 * pcp + k
                            nc.scalar.activation(Ew[:, k, :] if wave else Eown[:, pc, :],
                                                 psEk[:, :], AF.Relu)
                            nc.scalar.activation(Ow[:, k, :] if wave else Oown[:, pc, :],
                                                 psOk[:, :], AF.Relu)
                        csl = cos_t[:, p2, tsl]
                        snl = sin_t[:, p2, tsl]
                        t1 = work.tile([P, 2, HALF], f16, tag="rt1", bufs=1)
                        t2 = work.tile([P, 2, HALF], f16, tag="rt2", bufs=1)
                        nc.vector.tensor_mul(t1[:, :, :], Epair, csl)
                        nc.vector.tensor_mul(t2[:, :, :], Opair, snl)
                        nc.vector.tensor_sub(qrE[:, p2, tsl], t1[:, :, :], t2[:, :, :])
                        t3 = work.tile([P, 2, HALF], f16, tag="rt1", bufs=1, name="t3")
                        t4 = work.tile([P, 2, HALF], f16, tag="rt2", bufs=1, name="t4")
                        nc.vector.tensor_mul(t3[:, :, :], Opair, csl)
                        nc.vector.tensor_mul(t4[:, :, :], Epair, snl)
                        nc.vector.tensor_add(qrO[:, p2, tsl], t3[:, :, :], t4[:, :, :])
                        if pcp >= LAGP:
                            sc_mms(s_lo, ps_list, pcp - LAGP)
                    for pcp in range(NPC // 2 - LAGP, NPC // 2):
                        sc_mms(s_lo, ps_list, pcp)
                    for si in range(4):
                        s = s_lo + si
                        psS = ps_list[si]
                        if s < 4:
                            nc.vector.tensor_tensor(scT[:, s, s * P:(s + 1) * P],
                                                    psS[:, s * P:(s + 1) * P],
                                                    m0_t[:, :], OP.mult)
                            if s < 3:
                                nc.scalar.copy(scT[:, s, (s + 1) * P:HALF],
                                               psS[:, (s + 1) * P:HALF])
                        else:
                            nc.scalar.mul(scT[:, s, :], psS[:, :], gsel_t[:, 1:2])

                # ---- warm-keepers across the scT-copy -> ykv stall ----
                emit_dummies(DUMMY_KV, f"k{li}")

                # ---- ykv + LN + transpose ----
                for tb in range(4):
                    tbs = slice(tb * P, (tb + 1) * P)
                    psY = psp.tile([P, D], f32, tag="xspE", bufs=2)
                    for s in range(TBLK):
                        nc.tensor.matmul(psY[:, :], scT[:, s, tbs], x16[:, s, :],
                                         start=(s == 0), stop=(s == TBLK - 1))
                    r, nmr = ln_vecs(psY[:, :])
                    ykvn = work.tile([P, D], f16, tag="ykvn", bufs=2)
                    nc.scalar.activation(ykvn[:, :], psY[:, :], AF.Identity,
                                         bias=nmr[:, :], scale=r[:, :])
                    pe_transpose(ykvn[:, :], ykvT0, ykvT1, tb * P, f"y{li}_{tb}")

                # ---- y_sparse, xy, mlp partial (single pass) ----
                psM = []
                for mi in range(4):
                    psM.append(psp.tile([P, D], f32, tag=f"sc{mi}",
                                        name=f"psM{li}_{mi}"))
                for side in range(2):
                    wa, wb = (wye_a, wye_b) if side == 0 else (wyo_a, wyo_b)
                    own = Eown if side == 0 else Oown
                    enc_t = ence_t if side == 0 else enco_t
                    for pcp in range(NPC // 2):
                        ys = work.tile([P, 2, HALF], f16, tag="ys",
                                       name=f"ys{li}_{side}_{pcp}")
                        pcs0 = slice((2 * pcp) * P, (2 * pcp + 1) * P)
                        pcs1 = slice((2 * pcp + 1) * P, (2 * pcp + 2) * P)
                        psYS0 = psp.tile([P, HALF], f32, tag="xspO", bufs=2, name="psYS0")
                        psYS1 = psp.tile([P, HALF], f32, tag="xspO", bufs=2, name="psYS1")
                        nc.tensor.matmul(psYS0[:, :], wa[:, pcs0], ykvT0[:, :], start=True, stop=False)
                        nc.tensor.matmul(psYS1[:, :], wa[:, pcs1], ykvT0[:, :], start=True, stop=False)
                        nc.tensor.matmul(psYS0[:, :], wb[0:64, pcs0], ykvT1[0:64, :], start=False, stop=True)
                        nc.tensor.matmul(psYS1[:, :], wb[64:128, pcs1], ykvT1[64:128, :], start=False, stop=True)
                        nc.scalar.activation(ys[:, 0, :], psYS0[:, :], AF.Relu)
                        nc.scalar.activation(ys[:, 1, :], psYS1[:, :], AF.Relu)
                        xy = work.tile([P, 2, HALF], f16, tag="xy", bufs=2,
                                       name=f"xy{li}_{side}_{pcp}")
                        nc.vector.tensor_mul(xy[:, :, :],
                                             own[:, 2 * pcp:2 * pcp + 2, :],
                                             ys[:, :, :])
                        last = (side == 1 and pcp == NPC // 2 - 1)
                        for k in range(2):
                            pc = 2 * pcp + k
                            for tb in range(4):
                                nc.tensor.matmul(
                                    psM[tb][:, :],
                                    xy[:, k, tb * P:(tb + 1) * P],
                                    enc_t[:, pc, :],
                                    start=(side == 0 and pcp == 0 and k == 0),
                                    stop=(last and k == 1))

                # ---- masked scatter to canonical bounce + one AllReduce ----
                for tb in range(4):
                    bA = work.tile([P, D], f16, tag="bA", bufs=2)
                    bB = work.tile([P, D], f16, tag="bB", bufs=2)
                    nc.scalar.mul(bA[:, :], psM[tb][:, :], gsel_t[:, 0:1])
                    nc.scalar.mul(bB[:, :], psM[tb][:, :], gsel_t[:, 1:2])
                    nc.sync.dma_start(bounce_in[tb * P:(tb + 1) * P, :], bA[:, :])
                    nc.gpsimd.dma_start(bounce_in[HALF + tb * P:HALF + (tb + 1) * P, :], bB[:, :])
                nc.gpsimd.collective_compute(
                    "AllReduce", OP.add,
                    replica_groups=[list(range(NCORES))],
                    ins=[bounce_in.opt()],
                    outs=[bounce_out.opt()],
                )

                # ---- PE warm-keepers while the AR flies + LN runs ----
                emit_dummies(DUMMY_AR, f"d{li}")

                # ---- batched readback + LN + residual + new x16/xT ----
                tail_all(li)

            for rep in range(repeat):
                load_x_and_transpose()
                for li in range(N_LAYER):
                    layer(li)

            # ---- lm head (K=64 tails paired on row groups 0 / 64) ----
            for tbp in range(TBLK // 2):
                tb0, tb1 = 2 * tbp, 2 * tbp + 1
                tbs0 = slice(tb0 * P, (tb0 + 1) * P)
                tbs1 = slice(tb1 * P, (tb1 + 1) * P)
                psL0 = psp.tile([P, HALF], f32, tag="xspE", bufs=2, name="psL0")
                psL1 = psp.tile([P, HALF], f32, tag="xspO", bufs=2, name="psL1")
                nc.tensor.matmul(psL0[:, 0:VSH], xT0[:, tbs0], lmh_a[:, :], start=True, stop=False)
                nc.tensor.matmul(psL1[:, 0:VSH], xT0[:, tbs1], lmh_a[:, :], start=True, stop=False)
                nc.tensor.matmul(psL0[:, 0:VSH], xT1[0:64, tbs0], lmh_b[0:64, :], start=False, stop=True)
                nc.tensor.matmul(psL1[:, 0:VSH], xT1[64:128, tbs1], lmh_b[64:128, :], start=False, stop=True)
                for tb, tbs, psL in ((tb0, tbs0, psL0), (tb1, tbs1, psL1)):
                    outL = work.tile([P, VSH], f32, tag="outL", bufs=2)
                    nc.scalar.copy(outL[:, :], psL[:, 0:VSH])
                    (nc.sync if tb % 2 == 0 else nc.gpsimd).dma_start(logits_d[tbs, :], outL[:, :])

    nc.compile()
    _CACHE[key] = nc
    return nc


def make_inputs(idx, decoder_x, decoder_y, encoder, embed, pos_emb, lm_head):
    """Host-side prep: per-core input dicts (core c = head c//2, group c%2)."""
    idx = np.asarray(idx)
    decoder_x = np.asarray(decoder_x, dtype=np.float32)
    decoder_y = np.asarray(decoder_y, dtype=np.float32)
    encoder = np.asarray(encoder, dtype=np.float32).reshape(NH, N, D)
    embed = np.asarray(embed, dtype=np.float32)
    pos_emb = np.asarray(pos_emb, dtype=np.float32)
    lm_head = np.asarray(lm_head, dtype=np.float32)

    x0 = _ln_np(embed[idx[0]] + pos_emb[:T]).astype(np.float16)

    freqs = _get_freqs(N)
    fpair = freqs[0::2]
    tt = np.arange(T, dtype=np.float32)
    m0 = np.triu(np.ones((P, P), np.float32), k=1).astype(np.float16)
    lmh_full = lm_head.astype(np.float16)

    in_maps = []
    for c in range(NCORES):
        h, g = c // 2, c % 2
        tperm = tt if g == 0 else np.concatenate([tt[HALF:], tt[:HALF]])
        ph = ((fpair[:, None] * tperm[None, :]).astype(np.float32) % 1.0) \
            * np.float32(2.0 * math.pi)
        gsel = np.zeros((P, 2), np.float32)
        gsel[:, 0] = 1.0 if g == 0 else 0.0
        gsel[:, 1] = 1.0 - gsel[:, 0]
        x0c = x0 if g == 0 else np.concatenate([x0[HALF:], x0[:HALF]])
        in_maps.append({
            "x0": np.ascontiguousarray(x0c),
            "wxe": np.ascontiguousarray(decoder_x[h][:, 0::2]).astype(np.float16),
            "wxo": np.ascontiguousarray(decoder_x[h][:, 1::2]).astype(np.float16),
            "wye": np.ascontiguousarray(decoder_y[h][:, 0::2]).astype(np.float16),
            "wyo": np.ascontiguousarray(decoder_y[h][:, 1::2]).astype(np.float16),
            "ence": np.ascontiguousarray(encoder[h][0::2]).astype(np.float16),
            "enco": np.ascontiguousarray(encoder[h][1::2]).astype(np.float16),
            "cosT": np.cos(ph.astype(np.float64)).astype(np.float16),
            "sinT": np.sin(ph.astype(np.float64)).astype(np.float16),
            "m0": m0,
            "gsel": gsel,
            "lmh": np.ascontiguousarray(lmh_full[:, c * VSH:(c + 1) * VSH]),
            "ident": np.eye(P, dtype=np.float16),
        })
    return in_maps


def kernel(idx, decoder_x, decoder_y, encoder, embed, pos_emb, lm_head):
    from concourse.bass_utils import run_bass_kernel_spmd

    nc = build_program()
    in_maps = make_inputs(idx, decoder_x, decoder_y, encoder, embed, pos_emb,
                          lm_head)
    res = run_bass_kernel_spmd(nc, in_maps, list(range(NCORES)))
    return assemble_logits(res.results)


def assemble_logits(results):
    cols = []
    for c in range(NCORES):
        sl = results[c]["logits"]
        if c % 2 == 1:  # group-1 cores produce rows in swapped-half order
            sl = np.concatenate([sl[HALF:], sl[:HALF]], axis=0)
        cols.append(sl)
    logits = np.concatenate(cols, axis=1)
    return logits.reshape(1, T, VOCAB).astype(np.float32)
